# revision 1
# baseline (speedup 1.0000x reference)
"""Trainium2 Bass kernel for nn_CapsuleLayer (B=32, In=128, Din=256, ch=32, Nc=47, Dc=64).

Sharding: over the OUTPUT-CAPSULE axis Nc (47 -> pad 48 = 8 cores x 6 capsules).
Routing is fully independent per (batch, output-capsule), and W (94 MiB) is the
dominant HBM tensor -- Nc-sharding reads W exactly once total instead of
replicating it 8x as batch-sharding would.

All device data is bf16 (PSUM accumulation f32): halves HBM traffic vs f32 and
unlocks the DVE's 2x packed mode plus 4x-faster PE matmuls.  Measured numpy
end-to-end quantization error ~4e-3 (gate is 2e-2).

Per-core layout: inputs_hat IH stored [p=(b,rr) 128 partitions, (c, k, n) free]
with n INNERMOST -- so the E-weighted multiply (E broadcast over k, a middle
axis) still has unit-stride innermost reads and packs at 2x.  Produced by
per-channel matmuls ih_c[(b,rr),(k,n)] = xt_c[d,(b,rr)]^T . wt_c[d,(k,n)].
Partition-group (rr) reductions run on the PE with block-diagonal ones
matrices (BD4 [128,32], BD4T [32,128]); channel reductions accumulate on the
PE across per-channel matmuls; the k-reduction of the routing a-step uses
log2(64)=6 tree-halving tensor_adds (TensorReduce never packs; the tree does).

Routing iteration t:
  a   = sum_k OUT_{t-1} * IH        (DVE packed mul + 6 tree adds)
  E   = exp(sum_t a)                (ACT; softmax normalization folded into s)
  s   = (sum_i E*IH)/Z + B          (DVE packed muls in 4 c-chunks, PE
                                     BD4-stationary accumulating matmuls
                                     trailing each chunk)
  OUT = squash(s)                   (small [32,384] ops)
Iteration 1 (uniform c) accumulates s1 = BD4^T IH over channels on the PE
during phase 1.

Toolchain constraint: matmul (S3_LW) and DMA (DIRECT2D) instructions accept at
most ONE sync wait at codegen; DVE likewise.  The x|w stream is 8 one-shot
DMAs into disjoint regions of one resident SBUF tile, all on the SP HWDGE
queue (one semaphore, progressive arrival, serialized = real per-core HBM
bandwidth); both matmul operands come from the same stream region (one DMA
sem).  Const DMAs are pre-absorbed into the PE/DVE clocks; psum-slot WAR
ticks are absorbed via tiny dummy matmuls.
"""

import numpy as np
import ml_dtypes

B, IN, DIN = 32, 128, 256
CH, NC, DC = 32, 47, 64
NCP = 48          # padded Nc
NSH = 6           # capsules per core
NCORES = 8
NK = NSH * DC     # 384
EPS = 1e-7
SMUL_CHUNKS = (6, 7, 7, 6)  # DVE s-step c-chunks (balanced; swept optimum)
POOLC = 6                  # channels the Pool engine takes in the a/s-steps
DVEC = CH - POOLC          # 26 channels on the DVE

_cache = {}


def _build_nc():
    import concourse.bass as bass
    import concourse.tile as tile
    from concourse import mybir
    from concourse.tile_rust import add_dep_helper

    f32 = mybir.dt.float32
    bf16 = mybir.dt.bfloat16
    nc = bass.Bass()

    # packed stream: per channel rows
    # [xt0(128) | xt1(128) | wt0(384) | wt1(384) | xs0(32) | xs1(32)]
    # xs = sum_rr x (host-precomputed) lets the iter-1 s accumulate straight
    # from the stream (s1 = sum_{c,d} xs.W), taking the PSUM->SBUF copies off
    # the phase-1 critical path.
    xw = nc.dram_tensor("xw", [CH, 128, 1088], bf16, kind="ExternalInput")
    # consts: [bd4(0:32) | bd4t(rows0:32, 32:160) | brepR(160:544) |
    #          32*brepR(544:928)]  (brepR = B_param bcast to 128 partitions;
    # B_param enters s as Z*brep accumulated INTO the PSUM s-group, so the
    # squash can run straight off PSUM)
    cstb = nc.dram_tensor("cstb", [128, 928], bf16, kind="ExternalInput")
    out_d = nc.dram_tensor("out", [B, NK], f32, kind="ExternalOutput")

    ADD = mybir.AluOpType.add
    MULT = mybir.AluOpType.mult
    AX = mybir.AxisListType.X
    AF = mybir.ActivationFunctionType

    NCHUNK = 8                 # stream DMA chunks
    CPC = CH // NCHUNK         # channels per chunk = 4

    with tile.TileContext(nc) as tc:
        with (
            tc.tile_pool(name="singles", bufs=1) as singles,
            tc.tile_pool(name="work", bufs=1) as work,
            tc.tile_pool(name="small", bufs=2) as small,
            tc.tile_pool(name="ps_ih", bufs=3, space="PSUM") as ps_ih,
            tc.tile_pool(name="ps_s1", bufs=1, space="PSUM") as ps_s1,
            tc.tile_pool(name="ps_rep", bufs=1, space="PSUM") as ps_rep,
            tc.tile_pool(name="ps_sm", bufs=1, space="PSUM") as ps_sm,
            tc.tile_pool(name="ps_s2", bufs=2, space="PSUM") as ps_s2,
        ):
            # consts ride the ACT HWDGE queue so they don't head-block the
            # SP queue that carries the big x|w stream
            cstb_t = singles.tile([128, 928], bf16)
            cb_dma = nc.scalar.dma_start(out=cstb_t[:], in_=cstb[:])
            bd4_t = cstb_t[:, 0:B]              # [128, 32]
            bd4t_t = cstb_t[0:B, B:B + 128]     # [32, 128]
            brepR_t = cstb_t[:, 160:160 + NK]   # B_param replicated to 128 p
            brep32R_t = cstb_t[:, 544:544 + NK]  # 32*B_param replicated
            eps_t = singles.tile([B, 1], f32)
            nc.vector.memset(eps_t[:], EPS)
            # DVE is 1-wait-limited: pre-observe the const-DMA sem (ZB
            # reads brepR much later)
            dve_scr = singles.tile([2, 2], bf16)
            nc.vector.tensor_copy(dve_scr[:], cstb_t[:2, :2])
            # Pool is 1-wait-limited too: pre-observe the const-DMA sem
            pool_scr = singles.tile([2, 2], bf16)
            nc.gpsimd.tensor_copy(pool_scr[:], cstb_t[:2, :2])

            # IH[(b,rr), c, k, n] bf16
            IH = singles.tile([128, CH, DC, NSH], bf16)
            # resident packed stream; 8 one-shot DMAs into disjoint regions
            STREAM = singles.tile([128, CH, 1088], bf16)

            # Absorb the const-DMA sems into the PE clock (dummy matmul) so
            # real matmuls carry a single wait.  The dummy target and pz
            # share one PSUM bank (disjoint column regions).
            psmall = ps_sm.tile([128, 24], f32, tag="dummy")
            pd = psmall[0:2, 0:2]
            last_dummy = nc.tensor.matmul(
                pd, cstb_t[:2, :2], cstb_t[:2, :2], start=True, stop=True,
                skip_group_check=True,
            )

            psum_s1 = ps_s1.tile([B, NK], f32)

            # ---------------- phase 1: inputs_hat + iter-1 s ----------------
            # Uneven chunks: a small first chunk starts the PE pipeline early
            # (and keeps the PE p-state ramp alive); small last chunks
            # shorten the post-DMA tail.
            CHUNKS = (2,) * 14 + (1,) * 4  # steady feed; finer tail chunks
            s_dmas = []
            q0 = 0
            for csz in CHUNKS:
                s_dmas.append(nc.sync.dma_start(
                    out=STREAM[:, q0:q0 + csz, :],
                    in_=xw[q0:q0 + csz].rearrange("q d f -> d q f"),
                ))
                q0 += csz
            # PE p-state warm-up: the tensor engine ramps 591->320->160
            # ns/384-row matmul over ~3us of sustained use.  Burn the
            # DMA-head dead time (kernel start -> first chunk lands) with
            # back-to-back N=384 matmuls on memset data so the real phase-1
            # matmuls start at full speed.
            WARM = singles.tile([128, NK], bf16)
            nc.vector.memset(WARM[:], 0.0)
            pwarm = ps_rep.tile([128, NK], f32, tag="rep")
            for w in range(5):
                nc.tensor.matmul(pwarm[:], WARM[:, 0:128], WARM[:],
                                 start=True, stop=True, skip_group_check=True)
            # pacing dummy off chunk0 bridges any remaining gap
            warm0 = nc.tensor.matmul(pd, cstb_t[:2, :2], cstb_t[:2, :2],
                                     start=True, stop=True,
                                     skip_group_check=True)
            add_dep_helper(warm0.ins, s_dmas[0].ins, sync=True,
                           reason="PE ramp pacing off chunk0")
            last_dummy = warm0
            # In*B_param enters the iter-1 s-group as its start matmul:
            # sum_{p in b} bd4 . 32*brepR = 128*brep
            nc.tensor.matmul(psum_s1[:], bd4_t[:], brep32R_t,
                             start=True, stop=False, skip_group_check=True)
            # iter-1 s accumulation straight from the stream: s1[b,(k,n)] =
            # sum_{c,d} xs[d,b] . wt[d,(k,n)].  Waits only on the DMA, not
            # the psum-WAR copy chain.  For the LAST channels the s1-mms are
            # emitted BEFORE those channels' ih groups, so they run at each
            # chunk's landing and psum_s1 completes right behind the final
            # chunk while the ih pipeline still drains under the squash.
            S1_SPLIT = 24

            def s1_mms(c):
                for dc in range(2):
                    nc.tensor.matmul(
                        psum_s1[:],
                        STREAM[:, c, 1024 + B * dc:1024 + B * (dc + 1)],
                        STREAM[:, c, 256 + NK * dc:256 + NK * (dc + 1)],
                        start=False,
                        stop=(c == CH - 1 and dc == 1),
                        skip_group_check=True,
                    )

            copy_insts = []
            for c in range(CH):
                if c < S1_SPLIT:
                    s1_mms(c)
                elif c == S1_SPLIT:
                    for ct in range(S1_SPLIT, CH):
                        s1_mms(ct)
                if c >= 3:
                    # absorb the psum-slot WAR tick (copy of c-3) right
                    # before the ih-mms -- after the s1-mms, so those never
                    # stall on the copy chain
                    dmy = nc.tensor.matmul(pd, bd4_t[:2, :2], bd4_t[:2, :2],
                                           start=True, stop=True,
                                           skip_group_check=True)
                    add_dep_helper(dmy.ins, copy_insts[c - 3].ins, sync=True,
                                   reason="absorb psum WAR tick on PE")
                    last_dummy = dmy
                psum_ih = ps_ih.tile([128, NK], f32, tag="ih")
                for dc in range(2):
                    mih = nc.tensor.matmul(
                        psum_ih[:],
                        STREAM[:, c, 128 * dc:128 * (dc + 1)],
                        STREAM[:, c, 256 + NK * dc:256 + NK * (dc + 1)],
                        start=(dc == 0), stop=(dc == 1),
                    )
                    if dc == 0:
                        add_dep_helper(mih.ins, last_dummy.ins, sync=False,
                                       reason="order dummy before matmul")
                ihc = IH[:, c].rearrange("p k n -> p (k n)")
                # spread PSUM->SBUF copies across DVE and ACT (GPSIMD
                # cannot access PSUM on real hardware)
                if c % 2 == 0:
                    copy_insts.append(nc.vector.tensor_copy(ihc, psum_ih[:]))
                else:
                    copy_insts.append(nc.scalar.copy(ihc, psum_ih[:]))

            _absn = [0]
            abs_scr = singles.tile([2, 128], f32)

            def absorb(eng, src_ap):
                """Tiny copy on `eng` reading src_ap: pre-observes the
                producer's sem so the next real op keeps a single wait.
                Each call gets a disjoint slice of one scratch tile --
                shared offsets would add WAW waits between absorbs."""
                i = _absn[0]
                _absn[0] += 1
                scr = abs_scr[:, 2 * i:2 * i + 2]
                if eng == "v":
                    return nc.vector.tensor_copy(scr, src_ap)
                if eng == "p":
                    return nc.gpsimd.tensor_copy(scr, src_ap)
                return nc.scalar.copy(scr, src_ap)

            def pace(src_inst):
                """Tiny dummy matmul dep'd on src_inst: keeps the PE p-state
                ramp alive through long DVE-only stretches (>4us idle
                resets the tensor engine to 2-3.7x slower rows)."""
                dmy = nc.tensor.matmul(pd, cstb_t[:2, :2], cstb_t[:2, :2],
                                       start=True, stop=True,
                                       skip_group_check=True)
                add_dep_helper(dmy.ins, src_inst.ins, sync=True,
                               reason="PE ramp pacing")
                return dmy

            rep_copies = [None]
            rep_tile = [None]

            def squash(pS, it, Rz=None, Rz2=None):
                """Squash straight off the PSUM s-accumulator pS [B, NK]
                (layout (k, n)), which already includes Z*B_param.  The 1/Z
                softmax normalization folds into the PSUM read itself:
                  Sb = pS/Z ; m2 = sum_k Sb^2 ; g = m2/((1+m2)sqrt(m2+eps))
                  OUT = Sb * g
                Rz = 1/Z tile (None -> imm 1/In for iter 1).
                PSUM gets exactly ONE reader (the Sb mul): PSUM can't feed
                two inputs of one op, and cross-engine PSUM reader chains
                cost an extra sync wait each (writer + previous reader).
                it<3: Sb and g are replicated to 128 partitions SEPARATELY
                (Sb-replicate overlaps the g-chain; OUTr = repSb*grep), so
                the next iteration starts ~3 small ops earlier.
                it==3: returns OUT [B, NK] f32 for the DMA."""
                # Sb bf16 copy of pS, UNNORMALIZED (single PE wait; mixing
                # in the fresh Rz here would need a 2nd wait).  The 1/Z
                # normalization rides the tiny [32,6] g-path instead.
                Sb = work.tile([B, NK], bf16, tag="Sb")
                sbw = nc.vector.tensor_copy(Sb[:], pS)
                if it < 3:
                    # PE replicate of Sb, overlapped with the g-chain below.
                    # Absorb the Sb write (and, from iter 2, the ps_rep
                    # bank's previous ACT reader) into the PE clock first so
                    # the matmul keeps a single wait.
                    pace(sbw)
                    if rep_copies[0] is not None:
                        dmy = nc.tensor.matmul(pd, cstb_t[:2, :2],
                                               cstb_t[:2, :2], start=True,
                                               stop=True,
                                               skip_group_check=True)
                        add_dep_helper(dmy.ins, rep_copies[0].ins, sync=True,
                                       reason="absorb ps_rep WAR tick")
                    pr = ps_rep.tile([128, NK], f32, tag="rep")
                    nc.tensor.matmul(pr[:], bd4t_t[:], Sb[:],
                                     start=True, stop=True)
                    repSb = work.tile([128, NK], bf16, tag="repSb%d" % it)
                    rep_copies[0] = nc.scalar.copy(repSb[:], pr[:])
                    rep_tile[0] = repSb
                Psq = work.tile([B, NK], f32, tag="Ssq")
                nc.vector.tensor_mul(Psq[:], Sb[:], Sb[:])
                q2 = small.tile([B, NSH], f32, tag="q2")
                mred = nc.vector.tensor_reduce(
                    q2[:],
                    Psq[:].rearrange("p (k n) -> p n k", n=NSH),
                    axis=AX, op=ADD,
                )
                pace(mred)
                m2 = small.tile([B, NSH], f32, tag="m2")
                if Rz2 is None:
                    nc.vector.tensor_scalar_mul(m2[:], q2[:], 1.0 / (IN * IN))
                else:
                    nc.vector.tensor_mul(m2[:], q2[:], Rz2[:])
                absorb("s", m2[:2, :2])          # ACT clock <- m2 (DVE)
                sq = small.tile([B, NSH], f32, tag="sq")
                nc.scalar.activation(sq[:], m2[:], AF.Sqrt, bias=eps_t[:])
                absorb("v", sq[:2, :2])          # DVE clock <- sq (ACT)
                den = small.tile([B, NSH], f32, tag="den")
                nc.vector.scalar_tensor_tensor(
                    out=den[:], in0=m2[:], scalar=1.0, in1=sq[:],
                    op0=ADD, op1=MULT,
                )
                rden = small.tile([B, NSH], f32, tag="rden")
                nc.vector.reciprocal(rden[:], den[:])
                g_ = small.tile([B, NSH], f32, tag="g")
                nc.vector.tensor_mul(g_[:], m2[:], rden[:])
                gg = small.tile([B, NSH], bf16, tag="gg")
                if Rz is None:
                    nc.vector.tensor_scalar_mul(gg[:], g_[:], 1.0 / IN)
                else:
                    nc.vector.tensor_mul(gg[:], g_[:], Rz[:])
                if it == 3:
                    OUT = work.tile([B, NK], f32, tag="out3")
                    ow = nc.vector.tensor_mul(
                        OUT[:].rearrange("p (k n) -> p k n", n=NSH),
                        Sb[:].rearrange("p (k n) -> p k n", n=NSH),
                        gg[:].rearrange("p (o n) -> p o n", o=1)
                            .broadcast_to([B, DC, NSH]),
                    )
                    return OUT, ow
                # gg -> 128 partitions (tiny matmul), OUTr = repSb * grep
                pg = psmall[:, 16:16 + NSH]
                pgm = nc.tensor.matmul(pg, bd4t_t[:], gg[:],
                                       start=True, stop=True)
                grep = small.tile([128, NSH], bf16, tag="grep%d" % it)
                with nc.allow_low_precision(reason="copy of bf16 gains"):
                    nc.vector.tensor_copy(grep[:], pg)
                absorb("v", repSb[:2, :2])       # DVE clock <- repSb (ACT)
                OUTr = work.tile([128, NK], bf16, tag="rep")
                ow = nc.vector.tensor_mul(
                    OUTr[:].rearrange("p (k n) -> p k n", n=NSH),
                    repSb[:].rearrange("p (k n) -> p k n", n=NSH),
                    grep[:].rearrange("p (o n) -> p o n", o=1)
                        .broadcast_to([128, DC, NSH]),
                )
                return OUTr, ow

            # ---------------- iter 1 ----------------
            OUTr, _ = squash(psum_s1[:], 1)

            TMP = work.tile([128, DVEC, DC, NSH], bf16, tag="TMP")
            TREE = []
            for l in range(5):
                tl = work.tile([128, CH, DC // (2 ** (l + 1)), NSH], bf16,
                               tag="T%d" % l, name="T%d" % l)
                TREE.append(tl)
            Aprev = None
            for it in (2, 3):
                # a-step: A = sum_k OUTr * IH   -> [128, CH, NSH]
                # DVE takes channels 0:26 (two halves so a pacing dummy can
                # land between them); the Pool engine takes 26:32 (its own
                # buffer, per-iteration tag so no cross-iteration WAR) and
                # feeds its slice of tree level 1 itself.
                absorb("p", rep_tile[0][:2, :2])  # Pool clock <- ACT
                TMPP = work.tile([128, POOLC, DC, NSH], bf16,
                                 tag="TMPP%d" % it, name="TMPP%d" % it)
                nc.gpsimd.tensor_mul(
                    TMPP[:].rearrange("p c k n -> p c (k n)"),
                    IH[:, DVEC:].rearrange("p c k n -> p c (k n)"),
                    OUTr[:].rearrange("p (o f) -> p o f", o=1)
                          .broadcast_to([128, POOLC, NK]),
                )
                l1p = nc.gpsimd.tensor_add(
                    TREE[0][:, DVEC:], TMPP[:, :, 0:DC // 2, :],
                    TMPP[:, :, DC // 2:DC, :])
                for h in range(2):
                    hc = DVEC // 2
                    amul = nc.vector.tensor_mul(
                        TMP[:, h * hc:(h + 1) * hc]
                            .rearrange("p c k n -> p c (k n)"),
                        IH[:, h * hc:(h + 1) * hc]
                            .rearrange("p c k n -> p c (k n)"),
                        OUTr[:].rearrange("p (o f) -> p o f", o=1)
                              .broadcast_to([128, hc, NK]),
                    )
                    pace(amul)
                src = TMP
                l1i = None
                for l in range(5):
                    half = DC // (2 ** (l + 1))
                    if l == 1:
                        # absorb Pool's L1 slice -- pinned AFTER our own L1
                        # so the scheduler can't hoist it into a DVE stall
                        ab = absorb("v", TREE[0][:2, DVEC, 0, :2])
                        add_dep_helper(ab.ins, l1i.ins, sync=False,
                                       reason="absorb after own L1")
                    if l == 0:
                        tadd = nc.vector.tensor_add(
                            TREE[0][:, 0:DVEC], src[:, :, 0:half, :],
                            src[:, :, half:2 * half, :])
                        l1i = tadd
                    else:
                        tadd = nc.vector.tensor_add(
                            TREE[l][:], src[:, :, 0:half, :],
                            src[:, :, half:2 * half, :])
                    if l < 3:
                        pace(tadd)
                    src = TREE[l]
                A = work.tile([128, CH, 1, NSH], bf16, tag="A%d" % it)
                nc.vector.tensor_add(A[:], src[:, :, 0:1, :], src[:, :, 1:2, :])
                if Aprev is None:
                    BL = A
                    Aprev = A
                else:
                    BL = work.tile([128, CH, 1, NSH], bf16, tag="BL")
                    nc.vector.tensor_add(BL[:], A[:], Aprev[:])
                # E = exp(BL)
                absorb("s", BL[:2, 0, 0, :2])     # ACT clock <- BL (DVE)
                E = work.tile([128, CH, NSH], bf16, tag="E%d" % it)
                eact = nc.scalar.activation(
                    E[:], BL[:].rearrange("p c o n -> p c (o n)"), AF.Exp)
                pace(eact)
                # s-step: TMP = E*IH per c-chunk; PE accumulates BD4^T TMP.
                # The Z chain (Zp reduce, pz matmul, reciprocal) rides in the
                # gaps between chunk muls -- off the critical path.
                absorb("v", E[:2, 0, :2])         # DVE clock <- E (ACT)
                pS = ps_s2.tile([B, NK], f32, tag="pS")
                # Pool's share of the E-mul (channels 26:32), started at
                # E-ready, runs parallel with the DVE chunks below
                absorb("p", E[:2, 0, :2])         # Pool clock <- E (ACT)
                smulp = nc.gpsimd.tensor_mul(
                    TMPP[:],
                    IH[:, DVEC:],
                    E[:, DVEC:]
                      .rearrange("p c (o n) -> p c o n", o=1)
                      .broadcast_to([128, POOLC, DC, NSH]),
                )
                c0 = 0
                mm_last = None
                for gi, csz in enumerate(SMUL_CHUNKS):
                    nc.vector.tensor_mul(
                        TMP[:, c0:c0 + csz],
                        IH[:, c0:c0 + csz],
                        E[:, c0:c0 + csz]
                          .rearrange("p c (o n) -> p c o n", o=1)
                          .broadcast_to([128, csz, DC, NSH]),
                    )
                    if gi == 0:
                        # Zp[p, n] = sum_c E (DVE, before the chunk-0 mms
                        # consume the PE)
                        Zp = small.tile([128, NSH], bf16, tag="Zp")
                        with nc.allow_low_precision(reason="sum of positives"):
                            nc.vector.tensor_reduce(
                                Zp[:],
                                E[:].rearrange("p c n -> p n c"),
                                axis=AX, op=ADD,
                            )
                    for c in range(c0, c0 + csz):
                        mm_last = nc.tensor.matmul(
                            pS[:], bd4_t[:],
                            TMP[:, c].rearrange("p k n -> p (k n)"),
                            start=(c == 0), stop=False,
                            skip_group_check=True,
                        )
                    if gi == 0:
                        # Z = BD4 reduction over rr (PE, behind the mms)
                        pz = psmall[0:B, 8:8 + NSH]
                        nc.tensor.matmul(pz, bd4_t[:], Zp[:],
                                         start=True, stop=True)
                        # ZB = Zp*brep: with sum_rr folded in by one extra
                        # accumulating matmul, B_param enters pS as Z*brep
                        ZB = work.tile([128, NK], bf16, tag="ZB%d" % it)
                        zb = nc.vector.tensor_mul(
                            ZB[:].rearrange("p (k n) -> p k n", n=NSH),
                            brepR_t.rearrange("p (k n) -> p k n", n=NSH),
                            Zp[:].rearrange("p (o n) -> p o n", o=1)
                                .broadcast_to([128, DC, NSH]),
                        )
                    elif gi == 1:
                        Zs = small.tile([B, NSH], f32, tag="Zs")
                        nc.vector.tensor_copy(Zs[:], pz)
                        Rz = small.tile([B, NSH], f32, tag="Rz")
                        nc.vector.reciprocal(Rz[:], Zs[:])
                        Rz2 = small.tile([B, NSH], f32, tag="Rz2")
                        nc.vector.tensor_mul(Rz2[:], Rz[:], Rz[:])
                    elif gi == len(SMUL_CHUNKS) - 2:
                        # pool-channel mms ride here, before the last DVE
                        # chunk's mms (the Pool mul finishes around now)
                        for cp in range(POOLC):
                            mm = nc.tensor.matmul(
                                pS[:], bd4_t[:],
                                TMPP[:, cp].rearrange("p k n -> p (k n)"),
                                start=False, stop=False,
                                skip_group_check=True,
                            )
                            if cp == 0:
                                add_dep_helper(mm.ins, smulp.ins, sync=True,
                                               reason="pool mms wait Pool mul")
                    c0 += csz
                # ZB closes the accumulation group so the channel mms above
                # never stall on it
                mm_last = nc.tensor.matmul(pS[:], bd4_t[:], ZB[:],
                                           start=False, stop=True,
                                           skip_group_check=True)
                add_dep_helper(mm_last.ins, zb.ins, sync=True,
                               reason="ZB matmul waits ZB mul")
                OUT, out_w = squash(pS[:], it, Rz=Rz, Rz2=Rz2)
                if it < 3:
                    OUTr = OUT
                else:
                    # The out-DMA always carries a HW-DGE completion-sem
                    # wait; pre-absorb the OUT data dep into the ACT engine
                    # (which issues the DMA) so it stays at one sync wait.
                    absorb("s", OUT[:2, :2])
                    o_dma = nc.scalar.dma_start(out=out_d[:], in_=OUT[:])
                    # Pre-absorb every final sem into the SYNC engine so the
                    # Tile kernel-tail drain needs <=1 wait (codegen limit).
                    f_scr = small.tile([2, 4], f32, tag="fin")
                    f_act = nc.scalar.copy(f_scr[:, 0:2], OUT[:2, :2])
                    f_dve = nc.vector.tensor_copy(f_scr[:, 2:4], OUT[:2, :2])
                    f_pe = pace(out_w)           # last PE op, after all dummies
                    for fin in (cb_dma, *s_dmas, mm_last, zb,
                                f_act, f_dve, f_pe, o_dma):
                        fnop = nc.sync.nop()
                        add_dep_helper(fnop.ins, fin.ins, sync=True,
                                       reason="absorb final sem for tail drain")

    return nc


def _pack_inputs(inputs, W, B_param):
    """Host-side shard + relayout + bf16 cast. Returns list of 8 in_maps."""
    bf = ml_dtypes.bfloat16
    inputs = np.ascontiguousarray(inputs, dtype=np.float32)
    W = np.ascontiguousarray(W, dtype=np.float32)
    B_param = np.ascontiguousarray(B_param, dtype=np.float32)

    Wp = np.zeros((CH, NCP, DC, DIN), dtype=np.float32)
    Wp[:, :NC] = W
    Bp = np.zeros((NCP, DC), dtype=np.float32)
    Bp[:NC] = B_param

    # xt[c, dc, dd, (b,rr)] = x[b, 4c+rr, 128dc+dd]
    x4 = inputs.reshape(B, CH, 4, 2, 128)           # b, c, rr, dc, dd
    xt = np.ascontiguousarray(
        x4.transpose(1, 3, 4, 0, 2)).reshape(CH, 2, 128, 128).astype(bf)
    # xs[c, dc, dd, b] = sum_rr x  (for the stream-side iter-1 s accumulate)
    xs = np.ascontiguousarray(
        x4.sum(axis=2).transpose(1, 2, 3, 0)).astype(bf)  # c, dc, dd, b
    bd4 = np.zeros((128, B), dtype=np.float32)
    bd4[np.arange(128), np.arange(128) // 4] = 1.0

    in_maps = []
    for core in range(NCORES):
        sl = slice(core * NSH, (core + 1) * NSH)
        # wt[c, dc, dd, (k, n)] = W[c, n, k, 128dc+dd]  (n innermost)
        w5 = Wp[:, sl].reshape(CH, NSH, DC, 2, 128)  # c n k dc dd
        wtc = np.ascontiguousarray(
            w5.transpose(0, 3, 4, 2, 1)).reshape(CH, 2, 128, NK).astype(bf)
        xwc = np.zeros((CH, 128, 1088), dtype=bf)
        xwc[:, :, 0:128] = xt[:, 0]
        xwc[:, :, 128:256] = xt[:, 1]
        xwc[:, :, 256:256 + NK] = wtc[:, 0]
        xwc[:, :, 256 + NK:256 + 2 * NK] = wtc[:, 1]
        xwc[:, :, 1024:1024 + B] = xs[:, 0]
        xwc[:, :, 1024 + B:1024 + 2 * B] = xs[:, 1]
        # brep[(k, n)] = Bp[n, k], replicated to all 128 partitions
        brep = np.ascontiguousarray(Bp[sl].T).reshape(1, NK)
        cstb = np.zeros((128, 928), dtype=np.float32)
        cstb[:, 0:B] = bd4
        cstb[0:B, B:B + 128] = bd4.T
        cstb[:, 160:160 + NK] = brep
        cstb[:, 544:544 + NK] = 32.0 * brep
        in_maps.append(dict(xw=xwc, cstb=cstb.astype(bf)))
    return in_maps


def _run(inputs, W, B_param, trace=False):
    from concourse.bass_utils import run_bass_kernel_spmd

    if "nc" not in _cache:
        _cache["nc"] = _build_nc()
    nc = _cache["nc"]
    in_maps = _pack_inputs(inputs, W, B_param)
    res = run_bass_kernel_spmd(nc, in_maps, core_ids=list(range(NCORES)),
                               trace=trace)
    # out[b, (k, n)] -> [b, n, k]
    outs = [r["out"].reshape(B, DC, NSH).transpose(0, 2, 1)
            for r in res.results]
    full = np.concatenate(outs, axis=1)[:, :NC, :]
    return np.ascontiguousarray(full.astype(np.float32)), res


def kernel(inputs, W, B_param):
    out, _ = _run(inputs, W, B_param, trace=False)
    return out



# revision 14
# speedup vs baseline: 1.2385x; 1.2385x over previous
"""Trainium2 Bass kernel for nn_CapsuleLayer (B=32, In=128, Din=256, ch=32, Nc=47, Dc=64).

Sharding: over the OUTPUT-CAPSULE axis Nc (47 -> pad 48 = 8 cores x 6 capsules).
Routing is fully independent per (batch, output-capsule), and W is the dominant
HBM tensor -- Nc-sharding reads W exactly once total.

v2 redesign (trace-driven), measured facts from the v1 trace:
  - DVE tensor_tensor with a broadcast operand runs at 2x (0.55 ns/elem);
    clean unit-stride adds run at 4x (0.3 ns/elem).  GPSIMD tensor ops run at
    3-4 ns/elem AND degrade concurrent DVE throughput -> Pool engine unused.
  - PE matmuls with M=128 out-partitions run ~325 ns per 384 cols (PSUM-write
    bound, 2 cyc/col); M<=64 runs ~159 ns.  Any matmul costs >=158 ns, so
    sem-absorb dummies are now engine_nops (~20 ns).
  - ACT exp<->sqrt table swaps cost 1.28 us each (5/run in v1).  sqrt now
    runs on the DVE (bit-trick rsqrt + 2 Newton steps); ACT holds exp forever.
  - SP DMA descriptor issue costs ~5 ns/row; v1's [d, q, f] DRAM layout made
    128*32 rows (21 us of SP issue).  DRAM is now chunk-contiguous per
    partition: 128 rows per transfer.

HBM traffic: W rides as fp8 e3m4 with per-channel absmax scales folded into
the bf16 x operand on the host ((x/s).(W*s) = x.W), halving the stream to
5.2 MiB/core.  Numpy end-to-end says rel_err ~1.4e-2 (gate 2e-2).

Phase 1: 8 chunk-pairs (x bf16 [128,4,256] | W fp8 [128,4,768]) on the SP
queue; per channel two accumulating matmuls (lhsT=xt bf16, moving=wt fp8)
-> psum -> PSUM->SBUF copies alternating DVE/ACT.  The iter-1 mean rides the
idle DVE as an f32 running sum SIH += IH_c, folded over rr by one f32 BD4
matmul at the end.

Routing iteration t (all big ops DVE, PE only for BD4 folds + replicates):
  a   = tree-reduce_k (OUTr * IH)     (2x mul + 4x tree adds)
  E   = exp(sum_t a)                  (ACT, table resident)
  s   = (sum_i E*IH)/Z + B            (2x muls in 4 c-chunks, BD4 matmuls
                                       trailing each chunk; Z via G4 matmul)
  OUT = squash(s)  on 128 replicated partitions (PE bd4t replicate first,
        then all smalls once; rsqrt = bit-trick + 2 NR on DVE)
"""

import numpy as np
import ml_dtypes

B, IN, DIN = 32, 128, 256
CH, NC, DC = 32, 47, 64
NCP = 48          # padded Nc
NSH = 6           # capsules per core
NCORES = 8
NK = NSH * DC     # 384
EPS = 1e-7
NCHUNK = 8
CPC = CH // NCHUNK  # 4 channels per chunk
FP8W = True       # W as fp8 e3m4 (False -> bf16 fallback)
MAGIC = 0x5f3759df

_cache = {}


def _build_nc():
    import concourse.bass as bass
    import concourse.tile as tile
    from concourse import mybir
    from concourse.tile_rust import add_dep_helper

    f32 = mybir.dt.float32
    bf16 = mybir.dt.bfloat16
    i32 = mybir.dt.int32
    w8 = mybir.dt.float8e3 if FP8W else bf16
    RB = 512 + (768 if FP8W else 1536)   # stream bytes per channel row
    WB = 384 if FP8W else 768            # W bytes per dc half
    nc = bass.Bass()

    sd = nc.dram_tensor("sd", [NCHUNK, 128, CPC * RB], mybir.dt.uint8,
                        kind="ExternalInput")
    # consts bf16: [bd4(0:32) | bd4t(rows0:32, 32:160) | G4(160:288) |
    #               brepR(288:672)]
    cstb = nc.dram_tensor("cstb", [128, 672], bf16, kind="ExternalInput")
    cst4 = nc.dram_tensor("cst4", [128, 32], f32, kind="ExternalInput")
    out_d = nc.dram_tensor("out", [B, NK], f32, kind="ExternalOutput")

    ADD = mybir.AluOpType.add
    MULT = mybir.AluOpType.mult
    SUB = mybir.AluOpType.subtract
    SHR = mybir.AluOpType.logical_shift_right
    AX = mybir.AxisListType.X
    AF = mybir.ActivationFunctionType

    with tile.TileContext(nc) as tc:
        with (
            tc.tile_pool(name="singles", bufs=1) as singles,
            tc.tile_pool(name="work", bufs=1) as work,
            tc.tile_pool(name="small", bufs=2) as small,
            tc.tile_pool(name="ps_ih", bufs=5, space="PSUM") as ps_ih,
            tc.tile_pool(name="ps_s1", bufs=1, space="PSUM") as ps_s1,
            tc.tile_pool(name="ps_rep", bufs=1, space="PSUM") as ps_rep,
            tc.tile_pool(name="ps_s2", bufs=1, space="PSUM") as ps_s2,
        ):
            cstb_t = singles.tile([128, 672], bf16)
            cb_dma = nc.scalar.dma_start(out=cstb_t[:], in_=cstb[:])
            cst4_t = singles.tile([128, 32], f32)
            c4_dma = nc.scalar.dma_start(out=cst4_t[:], in_=cst4[:])
            bd4_t = cstb_t[:, 0:B]               # [128, 32]
            bd4t_t = cstb_t[0:B, B:B + 128]      # [32, 128]
            g4_t = cstb_t[:, 160:288]            # [128, 128] block-diag ones
            brepR_t = cstb_t[:, 288:288 + NK]    # B_param bcast to 128 p
            brepB_t = cstb_t[0:B, 288:288 + NK]  # same, 32 partitions
            bd4f_t = cst4_t[:, 0:B]              # [128, 32] f32

            # int/float const tiles for the rsqrt bit-trick
            icst = singles.tile([128, 12], i32)
            nc.vector.memset(icst[:, 0:6], 1)
            nc.vector.memset(icst[:, 6:12], MAGIC)
            ones_i = icst[:, 0:6]
            magic_i = icst[:, 6:12]
            fcst = singles.tile([128, 6], f32)
            nc.vector.memset(fcst[:], 1.5)

            # engine pre-observes of the const DMAs (keeps later real ops at
            # one sync wait each)
            dve_scr = singles.tile([2, 4], bf16)
            nc.vector.tensor_copy(dve_scr[:, 0:2], cstb_t[:2, :2])
            # preload the exp ACT table during the DMA head
            exp_scr = singles.tile([2, 2], f32)
            nc.vector.memset(exp_scr[:], 0.0)
            nc.scalar.activation(exp_scr[:], exp_scr[:], AF.Exp)

            IH = singles.tile([128, CH, DC, NSH], bf16)   # (k, n) free layout
            SRAW = singles.tile([128, CH, RB], mybir.dt.uint8)
            SIH = singles.tile([128, NK], f32)

            # PE p-state warm-up during the DMA head
            WARM = singles.tile([128, NK], bf16)
            nc.vector.memset(WARM[:], 0.0)
            # single-generation PSUM tiles: cross-generation pool reuse
            # costs a same-engine WAW sem wait; same-tile region reuse is
            # engine-order (free)
            prep = ps_rep.tile([128, 392], f32)
            psb = ps_s2.tile([B, NK], f32)
            for _ in range(4):
                nc.tensor.matmul(prep[:, 0:NK], WARM[:, 0:128], WARM[:],
                                 start=True, stop=True, skip_group_check=True)

            # single-generation psum tile: s1 accumulator rows 0:32 cols
            # 0:384, Z folds at cols 384:390, pace-dummy corner 390:392
            # (a recycled-pool corner would create tile-generation cycles)
            psum_s1 = ps_s1.tile([128, 392], f32)
            pd = psum_s1[64:66, 390:392]  # matmul out base must be 0/32/64
            # PE observes of the const DMAs (engine instruction required --
            # a seq nop does not update the engine's observed sem levels)
            for cdma in (cb_dma, c4_dma):
                cd = nc.tensor.matmul(pd, WARM[:2, :2], WARM[:2, :2],
                                      start=True, stop=True,
                                      skip_group_check=True)
                add_dep_helper(cd.ins, cdma.ins, sync=True,
                               reason="PE observe const dma")

            _absn = [0]
            abs_scr = singles.tile([2, 96], f32)

            def absorb(eng, src_ap):
                """Tiny copy on `eng` reading src_ap: pre-observes the
                producer's sem so the next real op keeps a single wait.
                Disjoint slices per call (a shared slot would add WAW
                self-waits between absorbs)."""
                i = _absn[0]
                _absn[0] += 1
                scr = abs_scr[:, 2 * i:2 * i + 2]
                if eng == "v":
                    return nc.vector.tensor_copy(scr, src_ap)
                return nc.scalar.copy(scr, src_ap)

            # ---------------- phase 1: inputs_hat + iter-1 mean ----------
            # one uint8 DMA per 4-channel chunk (single sem per chunk; 128
            # descriptor rows).  IH copies all ride ACT; the per-chunk WAR
            # dummy absorbs the ACT copy sem so real matmuls carry only the
            # chunk-DMA wait (ps_ih bufs=5: channel c reuses slot of c-5,
            # whose copy the previous chunk's dummy already observed).
            copy_insts = []
            s_dmas = []
            for k in range(NCHUNK):
                s_dmas.append(nc.sync.dma_start(
                    out=SRAW[:, k * CPC:(k + 1) * CPC, :], in_=sd[k]))
                for c in range(k * CPC, (k + 1) * CPC):
                    if c % CPC == 0 and c >= 4:
                        dmy = nc.tensor.matmul(pd, WARM[:2, :2], WARM[:2, :2],
                                               start=True, stop=True,
                                               skip_group_check=True)
                        add_dep_helper(dmy.ins, copy_insts[c - 2].ins,
                                       sync=True, reason="absorb psum WAR")
                    psum_ih = ps_ih.tile([128, NK], f32, tag="ih")
                    for dc in range(2):
                        nc.tensor.matmul(
                            psum_ih[:],
                            SRAW[:, c, 256 * dc:256 * (dc + 1)].bitcast(bf16),
                            SRAW[:, c, 512 + WB * dc:512 + WB * (dc + 1)]
                                .bitcast(w8),
                            start=(dc == 0), stop=(dc == 1),
                        )
                    ihc = IH[:, c].rearrange("p k n -> p (k n)")
                    copy_insts.append(nc.scalar.copy(ihc, psum_ih[:]))
                    # iter-1 running mean on the otherwise-idle DVE (f32).
                    # A tiny DVE pre-observe of the ACT copy first: engines
                    # issue OOO (8-deep), so the add's ACT dep + own-chain
                    # dep would otherwise need two sync waits (codegen max 1)
                    if c == 0:
                        nc.vector.tensor_copy(SIH[:], psum_ih[:])
                    else:
                        absorb("v", ihc[:2, :2])
                        nc.vector.tensor_add(SIH[:], SIH[:], ihc)

            # fold rr on the PE: psum_s1[b] = sum_rr SIH[(b,rr)]  (f32 matmul)
            mm_s1 = nc.tensor.matmul(psum_s1[0:B, 0:NK], bd4f_t, SIH[:],
                                     start=True, stop=True,
                                     skip_group_check=True)


            def pace(src_inst):
                dmy = nc.tensor.matmul(pd, cstb_t[:2, :2], cstb_t[:2, :2],
                                       start=True, stop=True,
                                       skip_group_check=True)
                add_dep_helper(dmy.ins, src_inst.ins, sync=True,
                               reason="PE ramp pacing")
                return dmy

            rep_prev = [None]

            def squash(pS, it, Rz=None, Rz2=None):
                """Squash off the PSUM accumulator pS (holds Z*(s-B)+Z*B).
                it<3: replicate Sb to 128 partitions FIRST (PE bd4t), run all
                smalls on the replicated form -> OUT is born replicated.
                it==3: stay on 32 partitions, return f32 OUT for the DMA.
                1/Z softmax normalization folds in via Rz/Rz2 (None = Z=1).
                rsqrt(m2+eps) via bit-trick + 2 Newton steps, all DVE."""
                if it == 1:
                    Sb = work.tile([B, NK], bf16, tag="Sb1")
                    sbw = nc.vector.scalar_tensor_tensor(
                        out=Sb[:], in0=pS, scalar=1.0 / IN, in1=brepB_t,
                        op0=MULT, op1=ADD)
                else:
                    Sb = work.tile([B, NK], bf16, tag="Sb%d" % it)
                    sbw = nc.vector.tensor_copy(Sb[:], pS)
                if it < 3:
                    # SbR copy rides the DVE so the replicate matmul's WAR
                    # (vs the previous generation's reader) coalesces with
                    # its data wait into one DVE sem level
                    nc.tensor.matmul(prep[:, 0:NK], bd4t_t, Sb[:],
                                     start=True, stop=True,
                                     skip_group_check=True)
                    SbR = work.tile([128, NK], bf16, tag="SbR%d" % it)
                    rep_prev[0] = nc.vector.tensor_copy(SbR[:], prep[:, 0:NK])
                    S, P = SbR, 128
                else:
                    S, P = Sb, B
                Psq = work.tile([P, NK], bf16, tag="Psq%d" % it)
                with nc.allow_low_precision(reason="squares for norm"):
                    nc.vector.tensor_mul(Psq[:], S[:], S[:])
                q2 = small.tile([P, NSH], f32, tag="q2%d" % it)
                nc.vector.tensor_reduce(
                    q2[:], Psq[:].rearrange("p (k n) -> p n k", n=NSH),
                    axis=AX, op=ADD)
                if Rz2 is None:
                    m2 = q2
                else:
                    m2 = small.tile([P, NSH], f32, tag="m2%d" % it)
                    nc.vector.tensor_mul(m2[:], q2[:], Rz2[:])
                # u = rsqrt(m2 + eps): bit-trick seed + 2 Newton steps
                t_ = small.tile([P, NSH], f32, tag="t%d" % it)
                nc.vector.tensor_scalar_add(t_[:], m2[:], EPS)
                ti = t_[:].bitcast(i32)
                j_ = small.tile([P, NSH], i32, tag="j%d" % it)
                nc.vector.tensor_tensor(out=j_[:], in0=ti, in1=ones_i[0:P, :],
                                        op=SHR)
                y0i = small.tile([P, NSH], i32, tag="y0%d" % it)
                nc.vector.tensor_tensor(out=y0i[:], in0=magic_i[0:P, :],
                                        in1=j_[:], op=SUB)
                y0 = y0i[:].bitcast(f32)
                u = y0
                for nr_i in range(2):
                    ysq = small.tile([P, NSH], f32, tag="ys%d_%d" % (it, nr_i))
                    nc.vector.tensor_mul(ysq[:], u, u)
                    av = small.tile([P, NSH], f32, tag="av%d_%d" % (it, nr_i))
                    nc.vector.tensor_mul(av[:], t_[:], ysq[:])
                    h = small.tile([P, NSH], f32, tag="h%d_%d" % (it, nr_i))
                    nc.vector.scalar_tensor_tensor(
                        out=h[:], in0=av[:], scalar=-0.5, in1=fcst[0:P, :],
                        op0=MULT, op1=ADD)
                    un = small.tile([P, NSH], f32, tag="u%d_%d" % (it, nr_i))
                    nc.vector.tensor_mul(un[:], h[:], u)
                    u = un[:]
                v = small.tile([P, NSH], f32, tag="v%d" % it)
                nc.vector.tensor_scalar_add(v[:], m2[:], 1.0)
                rden = small.tile([P, NSH], f32, tag="rd%d" % it)
                nc.vector.reciprocal(rden[:], v[:])
                g1 = small.tile([P, NSH], f32, tag="g1%d" % it)
                nc.vector.tensor_mul(g1[:], m2[:], u)
                gdt = f32 if it == 3 else bf16
                gg = small.tile([P, NSH], gdt, tag="gg%d" % it)
                with nc.allow_low_precision(reason="gain copy"):
                    if Rz is None:
                        nc.vector.tensor_mul(gg[:], g1[:], rden[:])
                    else:
                        g2 = small.tile([P, NSH], f32, tag="g2%d" % it)
                        nc.vector.tensor_mul(g2[:], g1[:], rden[:])
                        nc.vector.tensor_mul(gg[:], g2[:], Rz[:])
                odt = f32 if it == 3 else bf16
                OUT = work.tile([P, NK], odt, tag="out%d" % it)
                ow = nc.vector.tensor_mul(
                    OUT[:].rearrange("p (k n) -> p k n", n=NSH),
                    S[:].rearrange("p (k n) -> p k n", n=NSH),
                    gg[:].rearrange("p (o n) -> p o n", o=1)
                        .broadcast_to([P, DC, NSH]),
                )
                return OUT, ow

            # ---------------- iter 1 ----------------
            OUTr, _ = squash(psum_s1[0:B, 0:NK], 1)

            TMP = work.tile([128, CH, DC, NSH], bf16, tag="TMP")
            TREE = []
            for l in range(5):
                tl = work.tile([128, CH, DC // (2 ** (l + 1)), NSH], bf16,
                               tag="T%d" % l, name="T%d" % l)
                TREE.append(tl)
            Aprev = None
            SMUL = (8, 8, 8, 8)
            for it in (2, 3):
                # ---- a-step: TMP = OUTr*IH, tree-reduce k -> A [128,(c,n)]
                for h in range(2):
                    amul = nc.vector.tensor_mul(
                        TMP[:, h * 16:(h + 1) * 16]
                            .rearrange("p c k n -> p c (k n)"),
                        IH[:, h * 16:(h + 1) * 16]
                            .rearrange("p c k n -> p c (k n)"),
                        OUTr[:].rearrange("p (o f) -> p o f", o=1)
                              .broadcast_to([128, 16, NK]),
                    )
                    pace(amul)
                src = TMP
                for l in range(5):
                    half = DC // (2 ** (l + 1))
                    tadd = nc.vector.tensor_add(
                        TREE[l][:], src[:, :, 0:half, :],
                        src[:, :, half:2 * half, :])
                    if l == 2:
                        pace(tadd)
                    src = TREE[l]
                A = work.tile([128, CH, 1, NSH], bf16, tag="A%d" % it)
                nc.vector.tensor_add(A[:], src[:, :, 0:1, :],
                                     src[:, :, 1:2, :])
                if Aprev is None:
                    BL = A
                    Aprev = A
                else:
                    BL = work.tile([128, CH, 1, NSH], bf16, tag="BL")
                    nc.vector.tensor_add(BL[:], A[:], Aprev[:])
                # ---- E = exp(BL) on ACT (table resident)
                E = work.tile([128, CH, NSH], bf16, tag="E%d" % it)
                eact = nc.scalar.activation(
                    E[:], BL[:].rearrange("p c o n -> p c (o n)"), AF.Exp)
                pace(eact)
                # ---- s-step: TMP = E*IH per chunk, PE accumulates BD4^T TMP
                absorb("v", E[:2, 0, :2])  # chunk-0 mul keeps 1 wait (TMP WAR)
                pS = psb
                c0 = 0
                for gi, csz in enumerate(SMUL):
                    nc.vector.tensor_mul(
                        TMP[:, c0:c0 + csz],
                        IH[:, c0:c0 + csz],
                        E[:, c0:c0 + csz]
                          .rearrange("p c (o n) -> p c o n", o=1)
                          .broadcast_to([128, csz, DC, NSH]),
                    )
                    if gi == 0:
                        # Zp[p, n] = sum_c E  (before chunk-0 mms claim PE)
                        Zp = small.tile([128, NSH], bf16, tag="Zp")
                        with nc.allow_low_precision(reason="sum of positives"):
                            nc.vector.tensor_reduce(
                                Zp[:], E[:].rearrange("p c n -> p n c"),
                                axis=AX, op=ADD)
                    for c in range(c0, c0 + csz):
                        nc.tensor.matmul(
                            pS[:], bd4_t,
                            TMP[:, c].rearrange("p k n -> p (k n)"),
                            start=(c == 0), stop=False,
                            skip_group_check=True,
                        )
                    if gi == 0:
                        # Z fold on the PE, replicated for it==2 (G4) or
                        # 32-partition for it==3 (bd4)
                        if it < 3:
                            pzap = prep[:, 384:390]
                            pzmm = nc.tensor.matmul(
                                pzap, g4_t, Zp[:],
                                start=True, stop=True, skip_group_check=True)
                            PZ = 128
                        else:
                            pzap = psum_s1[0:B, 384:390]
                            pzmm = nc.tensor.matmul(
                                pzap, bd4_t, Zp[:],
                                start=True, stop=True, skip_group_check=True)
                            PZ = B
                        ZB = work.tile([128, NK], bf16, tag="ZB%d" % it)
                        zb = nc.vector.tensor_mul(
                            ZB[:].rearrange("p (k n) -> p k n", n=NSH),
                            brepR_t.rearrange("p (k n) -> p k n", n=NSH),
                            Zp[:].rearrange("p (o n) -> p o n", o=1)
                                .broadcast_to([128, DC, NSH]),
                        )
                    elif gi == 1:
                        Zs = small.tile([PZ, NSH], f32, tag="Zs%d" % it)
                        nc.vector.tensor_copy(Zs[:], pzap)
                        Rz = small.tile([PZ, NSH], f32, tag="Rz%d" % it)
                        nc.vector.reciprocal(Rz[:], Zs[:])
                        Rz2 = small.tile([PZ, NSH], f32, tag="Rz2%d" % it)
                        nc.vector.tensor_mul(Rz2[:], Rz[:], Rz[:])
                    c0 += csz
                # ZB closes the accumulation group
                mm_last = nc.tensor.matmul(pS[:], bd4_t, ZB[:],
                                           start=False, stop=True,
                                           skip_group_check=True)
                add_dep_helper(mm_last.ins, zb.ins, sync=True,
                               reason="ZB matmul waits ZB mul")
                OUT, out_w = squash(pS[:], it, Rz=Rz, Rz2=Rz2)
                if it < 3:
                    OUTr = OUT
                else:
                    absorb("s", OUT[:2, :2])
                    o_dma = nc.scalar.dma_start(out=out_d[:], in_=OUT[:])
                    f_scr = small.tile([2, 4], f32, tag="fin")
                    f_act = nc.scalar.copy(f_scr[:, 0:2], OUT[:2, :2])
                    f_dve = nc.vector.tensor_copy(f_scr[:, 2:4], OUT[:2, :2])
                    f_pe = pace(out_w)
                    for fin in (cb_dma, c4_dma, *s_dmas, mm_last, mm_s1,
                                zb, f_act, f_dve, f_pe, o_dma):
                        fnop = nc.sync.nop()
                        add_dep_helper(fnop.ins, fin.ins, sync=True,
                                       reason="absorb final sem for drain")

    return nc


def _pack_inputs(inputs, W, B_param):
    bf = ml_dtypes.bfloat16
    w8 = ml_dtypes.float8_e3m4 if FP8W else bf
    inputs = np.ascontiguousarray(inputs, dtype=np.float32)
    W = np.ascontiguousarray(W, dtype=np.float32)
    B_param = np.ascontiguousarray(B_param, dtype=np.float32)

    Wp = np.zeros((CH, NCP, DC, DIN), dtype=np.float32)
    Wp[:, :NC] = W
    Bp = np.zeros((NCP, DC), dtype=np.float32)
    Bp[:NC] = B_param

    # xt[c, dc, dd, (b,rr)] = x[b, 4c+rr, 128dc+dd]
    x4 = inputs.reshape(B, CH, 4, 2, 128)            # b c rr dc dd
    xt = np.ascontiguousarray(
        x4.transpose(1, 3, 4, 0, 2)).reshape(CH, 2, 128, 128)
    bd4 = np.zeros((128, B), dtype=np.float32)
    bd4[np.arange(128), np.arange(128) // 4] = 1.0
    g4 = np.zeros((128, 128), dtype=np.float32)
    g4[np.arange(128)[:, None] // 4 == np.arange(128)[None, :] // 4] = 1.0

    in_maps = []
    for core in range(NCORES):
        sl = slice(core * NSH, (core + 1) * NSH)
        # wt[c, dc, dd, (k, n)] = W[c, n, k, 128dc+dd]
        w5 = Wp[:, sl].reshape(CH, NSH, DC, 2, 128)  # c n k dc dd
        wt = np.ascontiguousarray(
            w5.transpose(0, 3, 4, 2, 1)).reshape(CH, 2, 128, NK)
        if FP8W:
            amax = np.abs(wt).reshape(CH, -1).max(axis=1)
            sw = 15.0 / np.maximum(amax, 1e-30)
        else:
            sw = np.ones(CH, dtype=np.float32)
        wt_q = (wt * sw[:, None, None, None]).astype(w8)
        xt_c = (xt / sw[:, None, None, None]).astype(bf)
        # merged byte stream [c, dd, xt0|xt1|wt0|wt1], chunk-contiguous
        RB = 512 + (768 if FP8W else 1536)
        WBY = 384 if FP8W else 768
        sb = np.zeros((CH, 128, RB), dtype=np.uint8)
        xb = np.ascontiguousarray(xt_c.transpose(0, 2, 1, 3))  # c dd dc br
        sb[:, :, 0:512] = xb.view(np.uint8).reshape(CH, 128, 512)
        wb = np.ascontiguousarray(wt_q.transpose(0, 2, 1, 3))  # c dd dc kn
        sb[:, :, 512:RB] = wb.view(np.uint8).reshape(CH, 128, 2 * WBY)
        sdc = np.ascontiguousarray(
            sb.reshape(NCHUNK, CPC, 128, RB).transpose(0, 2, 1, 3)
        ).reshape(NCHUNK, 128, CPC * RB)
        brep = np.ascontiguousarray(Bp[sl].T).reshape(1, NK)  # (k, n) flat
        cstb = np.zeros((128, 672), dtype=np.float32)
        cstb[:, 0:B] = bd4
        cstb[0:B, B:B + 128] = bd4.T
        cstb[:, 160:288] = g4
        cstb[:, 288:288 + NK] = brep
        cst4 = np.zeros((128, 32), dtype=np.float32)
        cst4[:, 0:B] = bd4
        in_maps.append(dict(sd=sdc, cstb=cstb.astype(bf), cst4=cst4))
    return in_maps


def _run(inputs, W, B_param, trace=False):
    from concourse.bass_utils import run_bass_kernel_spmd

    if "nc" not in _cache:
        _cache["nc"] = _build_nc()
    nc = _cache["nc"]
    in_maps = _pack_inputs(inputs, W, B_param)
    res = run_bass_kernel_spmd(nc, in_maps, core_ids=list(range(NCORES)),
                               trace=trace)
    # out[b, (k, n)] -> [b, n, k]
    outs = [r["out"].reshape(B, DC, NSH).transpose(0, 2, 1)
            for r in res.results]
    full = np.concatenate(outs, axis=1)[:, :NC, :]
    return np.ascontiguousarray(full.astype(np.float32)), res


def kernel(inputs, W, B_param):
    out, _ = _run(inputs, W, B_param, trace=False)
    return out


# revision 15
# speedup vs baseline: 1.3111x; 1.0586x over previous
"""Trainium2 Bass kernel for nn_CapsuleLayer (B=32, In=128, Din=256, ch=32, Nc=47, Dc=64).

Sharding: over the OUTPUT-CAPSULE axis Nc (47 -> pad 48 = 8 cores x 6 capsules).
Routing is fully independent per (batch, output-capsule), and W is the dominant
HBM tensor -- Nc-sharding reads W exactly once total.

v2 redesign (trace-driven), measured facts from the v1 trace:
  - DVE tensor_tensor with a broadcast operand runs at 2x (0.55 ns/elem);
    clean unit-stride adds run at 4x (0.3 ns/elem).  GPSIMD tensor ops run at
    3-4 ns/elem AND degrade concurrent DVE throughput -> Pool engine unused.
  - PE matmuls with M=128 out-partitions run ~325 ns per 384 cols (PSUM-write
    bound, 2 cyc/col); M<=64 runs ~159 ns.  Any matmul costs >=158 ns, so
    sem-absorb dummies are now engine_nops (~20 ns).
  - ACT exp<->sqrt table swaps cost 1.28 us each (5/run in v1).  sqrt now
    runs on the DVE (bit-trick rsqrt + 2 Newton steps); ACT holds exp forever.
  - SP DMA descriptor issue costs ~5 ns/row; v1's [d, q, f] DRAM layout made
    128*32 rows (21 us of SP issue).  DRAM is now chunk-contiguous per
    partition: 128 rows per transfer.

HBM traffic: W rides as fp8 e3m4 with per-channel absmax scales folded into
the bf16 x operand on the host ((x/s).(W*s) = x.W), halving the stream to
5.2 MiB/core.  Numpy end-to-end says rel_err ~1.4e-2 (gate 2e-2).

Phase 1: 8 chunk-pairs (x bf16 [128,4,256] | W fp8 [128,4,768]) on the SP
queue; per channel two accumulating matmuls (lhsT=xt bf16, moving=wt fp8)
-> psum -> PSUM->SBUF copies alternating DVE/ACT.  The iter-1 mean rides the
idle DVE as an f32 running sum SIH += IH_c, folded over rr by one f32 BD4
matmul at the end.

Routing iteration t (all big ops DVE, PE only for BD4 folds + replicates):
  a   = tree-reduce_k (OUTr * IH)     (2x mul + 4x tree adds)
  E   = exp(sum_t a)                  (ACT, table resident)
  s   = (sum_i E*IH)/Z + B            (2x muls in 4 c-chunks, BD4 matmuls
                                       trailing each chunk; Z via G4 matmul)
  OUT = squash(s)  on 128 replicated partitions (PE bd4t replicate first,
        then all smalls once; rsqrt = bit-trick + 2 NR on DVE)
"""

import numpy as np
import ml_dtypes

B, IN, DIN = 32, 128, 256
CH, NC, DC = 32, 47, 64
NCP = 48          # padded Nc
NSH = 6           # capsules per core
NCORES = 8
NK = NSH * DC     # 384
EPS = 1e-7
NCHUNK = 8
CPC = CH // NCHUNK  # 4 channels per chunk
FP8W = True       # W as fp8 e3m4 (False -> bf16 fallback)
MAGIC = 0x5f3759df

_cache = {}


def _build_nc():
    import concourse.bass as bass
    import concourse.tile as tile
    from concourse import mybir
    from concourse.tile_rust import add_dep_helper

    f32 = mybir.dt.float32
    bf16 = mybir.dt.bfloat16
    i32 = mybir.dt.int32
    w8 = mybir.dt.float8e3 if FP8W else bf16
    RB = 512 + (768 if FP8W else 1536)   # stream bytes per channel row
    WB = 384 if FP8W else 768            # W bytes per dc half
    nc = bass.Bass()

    sd = nc.dram_tensor("sd", [NCHUNK, 128, CPC * RB], mybir.dt.uint8,
                        kind="ExternalInput")
    # consts bf16: [bd4(0:32) | bd4t(rows0:32, 32:160) | G4(160:288) |
    #               brepR(288:672)]
    cstb = nc.dram_tensor("cstb", [128, 672], bf16, kind="ExternalInput")
    out_d = nc.dram_tensor("out", [B, NK], f32, kind="ExternalOutput")

    ADD = mybir.AluOpType.add
    MULT = mybir.AluOpType.mult
    SUB = mybir.AluOpType.subtract
    SHR = mybir.AluOpType.logical_shift_right
    AX = mybir.AxisListType.X
    AF = mybir.ActivationFunctionType

    with tile.TileContext(nc) as tc:
        with (
            tc.tile_pool(name="singles", bufs=1) as singles,
            tc.tile_pool(name="work", bufs=1) as work,
            tc.tile_pool(name="small", bufs=2) as small,
            tc.tile_pool(name="ps_ih", bufs=5, space="PSUM") as ps_ih,
            tc.tile_pool(name="ps_s1", bufs=1, space="PSUM") as ps_s1,
            tc.tile_pool(name="ps_rep", bufs=1, space="PSUM") as ps_rep,
            tc.tile_pool(name="ps_s2", bufs=1, space="PSUM") as ps_s2,
        ):
            cstb_t = singles.tile([128, 672], bf16)
            cb_dma = nc.scalar.dma_start(out=cstb_t[:], in_=cstb[:])
            bd4_t = cstb_t[:, 0:B]               # [128, 32]
            bd4t_t = cstb_t[0:B, B:B + 128]      # [32, 128]
            g4_t = cstb_t[:, 160:288]            # [128, 128] block-diag ones
            brepR_t = cstb_t[:, 288:288 + NK]    # B_param bcast to 128 p
            brepB_t = cstb_t[0:B, 288:288 + NK]  # same, 32 partitions

            # int/float const tiles for the rsqrt bit-trick
            icst = singles.tile([128, 12], i32)
            nc.vector.memset(icst[:, 0:6], 1)
            nc.vector.memset(icst[:, 6:12], MAGIC)
            ones_i = icst[:, 0:6]
            magic_i = icst[:, 6:12]
            fcst = singles.tile([128, 6], f32)
            nc.vector.memset(fcst[:], 1.5)

            # engine pre-observes of the const DMAs (keeps later real ops at
            # one sync wait each)
            dve_scr = singles.tile([2, 4], bf16)
            nc.vector.tensor_copy(dve_scr[:, 0:2], cstb_t[:2, :2])
            # preload the exp ACT table during the DMA head
            exp_scr = singles.tile([2, 2], f32)
            nc.vector.memset(exp_scr[:], 0.0)
            nc.scalar.activation(exp_scr[:], exp_scr[:], AF.Exp)

            IH = singles.tile([128, CH, DC, NSH], bf16)   # (k, n) free layout
            SRAW = singles.tile([128, CH, RB], mybir.dt.uint8)
            # iter-1 mean: bf16 pairwise tree (f32 serial chain is 1x DVE and
            # lags the phase-1 window; pairwise also needs no sem absorbs --
            # both inputs of a P-add come off the same ACT copy sem)
            PT = singles.tile([128, 16, NK], bf16)
            QT = singles.tile([128, 8, NK], bf16)
            RT = singles.tile([128, 4, NK], bf16)
            ST = singles.tile([128, 2, NK], bf16)
            SIH = singles.tile([128, NK], bf16)

            # PE p-state warm-up during the DMA head
            WARM = singles.tile([128, NK], bf16)
            nc.vector.memset(WARM[:], 0.0)
            # single-generation PSUM tiles: cross-generation pool reuse
            # costs a same-engine WAW sem wait; same-tile region reuse is
            # engine-order (free)
            prep = ps_rep.tile([128, 392], f32)
            psb = ps_s2.tile([B, NK], f32)
            for _ in range(4):
                nc.tensor.matmul(prep[:, 0:NK], WARM[:, 0:128], WARM[:],
                                 start=True, stop=True, skip_group_check=True)

            # single-generation psum tile: s1 accumulator rows 0:32 cols
            # 0:384, Z folds at cols 384:390, pace-dummy corner 390:392
            # (a recycled-pool corner would create tile-generation cycles)
            psum_s1 = ps_s1.tile([128, 392], f32)
            pd = psum_s1[64:66, 390:392]  # matmul out base must be 0/32/64
            # PE observes of the const DMAs (engine instruction required --
            # a seq nop does not update the engine's observed sem levels)
            cd = nc.tensor.matmul(pd, WARM[:2, :2], WARM[:2, :2],
                                  start=True, stop=True,
                                  skip_group_check=True)
            add_dep_helper(cd.ins, cb_dma.ins, sync=True,
                           reason="PE observe const dma")

            _absn = [0]
            abs_scr = singles.tile([2, 96], f32)

            def absorb(eng, src_ap):
                """Tiny copy on `eng` reading src_ap: pre-observes the
                producer's sem so the next real op keeps a single wait.
                Disjoint slices per call (a shared slot would add WAW
                self-waits between absorbs)."""
                i = _absn[0]
                _absn[0] += 1
                scr = abs_scr[:, 2 * i:2 * i + 2]
                if eng == "v":
                    return nc.vector.tensor_copy(scr, src_ap)
                return nc.scalar.copy(scr, src_ap)

            # ---------------- phase 1: inputs_hat + iter-1 mean ----------
            # one uint8 DMA per 4-channel chunk (single sem per chunk; 128
            # descriptor rows).  IH copies all ride ACT; the per-chunk WAR
            # dummy absorbs the ACT copy sem so real matmuls carry only the
            # chunk-DMA wait (ps_ih bufs=5: channel c reuses slot of c-5,
            # whose copy the previous chunk's dummy already observed).
            copy_insts = []
            s_dmas = []
            for k in range(NCHUNK):
                s_dmas.append(nc.sync.dma_start(
                    out=SRAW[:, k * CPC:(k + 1) * CPC, :], in_=sd[k]))
                for c in range(k * CPC, (k + 1) * CPC):
                    if c % CPC == 0 and c >= 4:
                        dmy = nc.tensor.matmul(pd, WARM[:2, :2], WARM[:2, :2],
                                               start=True, stop=True,
                                               skip_group_check=True)
                        add_dep_helper(dmy.ins, copy_insts[c - 2].ins,
                                       sync=True, reason="absorb psum WAR")
                    psum_ih = ps_ih.tile([128, NK], f32, tag="ih")
                    for dc in range(2):
                        nc.tensor.matmul(
                            psum_ih[:],
                            SRAW[:, c, 256 * dc:256 * (dc + 1)].bitcast(bf16),
                            SRAW[:, c, 512 + WB * dc:512 + WB * (dc + 1)]
                                .bitcast(w8),
                            start=(dc == 0), stop=(dc == 1),
                        )
                    ihc = IH[:, c].rearrange("p k n -> p (k n)")
                    copy_insts.append(nc.scalar.copy(ihc, psum_ih[:]))
                    with nc.allow_low_precision(reason="iter-1 mean tree"):
                        if c % 2 == 1:
                            nc.vector.tensor_add(
                                PT[:, c // 2],
                                IH[:, c - 1].rearrange("p k n -> p (k n)"),
                                ihc)
                        if c % 4 == 3:
                            nc.vector.tensor_add(
                                QT[:, c // 4], PT[:, c // 2 - 1], PT[:, c // 2])
                        if c % 8 == 7:
                            nc.vector.tensor_add(
                                RT[:, c // 8], QT[:, c // 4 - 1], QT[:, c // 4])
                        if c % 16 == 15:
                            nc.vector.tensor_add(
                                ST[:, c // 16], RT[:, c // 8 - 1], RT[:, c // 8])
                        if c == CH - 1:
                            nc.vector.tensor_add(SIH[:], ST[:, 0], ST[:, 1])

            # fold rr on the PE: psum_s1[b] = sum_rr SIH[(b,rr)]
            mm_s1 = nc.tensor.matmul(psum_s1[0:B, 0:NK], bd4_t, SIH[:],
                                     start=True, stop=True,
                                     skip_group_check=True)


            def pace(src_inst):
                dmy = nc.tensor.matmul(pd, cstb_t[:2, :2], cstb_t[:2, :2],
                                       start=True, stop=True,
                                       skip_group_check=True)
                add_dep_helper(dmy.ins, src_inst.ins, sync=True,
                               reason="PE ramp pacing")
                return dmy

            rep_prev = [None]

            def squash(pS, it, Rz=None, Rz2=None):
                """Squash off the PSUM accumulator pS (holds Z*(s-B)+Z*B).
                it<3: replicate Sb to 128 partitions FIRST (PE bd4t), run all
                smalls on the replicated form -> OUT is born replicated.
                it==3: stay on 32 partitions, return f32 OUT for the DMA.
                1/Z softmax normalization folds in via Rz/Rz2 (None = Z=1).
                rsqrt(m2+eps) via bit-trick + 2 Newton steps, all DVE."""
                if it == 1:
                    Sb = work.tile([B, NK], bf16, tag="Sb1")
                    sbw = nc.vector.scalar_tensor_tensor(
                        out=Sb[:], in0=pS, scalar=1.0 / IN, in1=brepB_t,
                        op0=MULT, op1=ADD)
                else:
                    Sb = work.tile([B, NK], bf16, tag="Sb%d" % it)
                    sbw = nc.vector.tensor_copy(Sb[:], pS)
                pace(sbw)
                if it < 3:
                    # SbR copy rides the DVE so the replicate matmul's WAR
                    # (vs the previous generation's reader) coalesces with
                    # its data wait into one DVE sem level
                    nc.tensor.matmul(prep[:, 0:NK], bd4t_t, Sb[:],
                                     start=True, stop=True,
                                     skip_group_check=True)
                    SbR = work.tile([128, NK], bf16, tag="SbR%d" % it)
                    rep_prev[0] = nc.vector.tensor_copy(SbR[:], prep[:, 0:NK])
                    S, P = SbR, 128
                else:
                    S, P = Sb, B
                Psq = work.tile([P, NK], bf16, tag="Psq%d" % it)
                with nc.allow_low_precision(reason="squares for norm"):
                    nc.vector.tensor_mul(Psq[:], S[:], S[:])
                q2 = small.tile([P, NSH], f32, tag="q2%d" % it)
                q2r = nc.vector.tensor_reduce(
                    q2[:], Psq[:].rearrange("p (k n) -> p n k", n=NSH),
                    axis=AX, op=ADD)
                pace(q2r)
                if Rz2 is None:
                    m2 = q2
                else:
                    m2 = small.tile([P, NSH], f32, tag="m2%d" % it)
                    nc.vector.tensor_mul(m2[:], q2[:], Rz2[:])
                # u = rsqrt(m2 + eps): bit-trick seed + 2 Newton steps
                t_ = small.tile([P, NSH], f32, tag="t%d" % it)
                nc.vector.tensor_scalar_add(t_[:], m2[:], EPS)
                ti = t_[:].bitcast(i32)
                j_ = small.tile([P, NSH], i32, tag="j%d" % it)
                nc.vector.tensor_tensor(out=j_[:], in0=ti, in1=ones_i[0:P, :],
                                        op=SHR)
                y0i = small.tile([P, NSH], i32, tag="y0%d" % it)
                nc.vector.tensor_tensor(out=y0i[:], in0=magic_i[0:P, :],
                                        in1=j_[:], op=SUB)
                y0 = y0i[:].bitcast(f32)
                u = y0
                for nr_i in range(1):
                    ysq = small.tile([P, NSH], f32, tag="ys%d_%d" % (it, nr_i))
                    nc.vector.tensor_mul(ysq[:], u, u)
                    av = small.tile([P, NSH], f32, tag="av%d_%d" % (it, nr_i))
                    nc.vector.tensor_mul(av[:], t_[:], ysq[:])
                    h = small.tile([P, NSH], f32, tag="h%d_%d" % (it, nr_i))
                    nc.vector.scalar_tensor_tensor(
                        out=h[:], in0=av[:], scalar=-0.5, in1=fcst[0:P, :],
                        op0=MULT, op1=ADD)
                    un = small.tile([P, NSH], f32, tag="u%d_%d" % (it, nr_i))
                    nc.vector.tensor_mul(un[:], h[:], u)
                    u = un[:]
                v = small.tile([P, NSH], f32, tag="v%d" % it)
                nc.vector.tensor_scalar_add(v[:], m2[:], 1.0)
                rden = small.tile([P, NSH], f32, tag="rd%d" % it)
                nc.vector.reciprocal(rden[:], v[:])
                g1 = small.tile([P, NSH], f32, tag="g1%d" % it)
                nc.vector.tensor_mul(g1[:], m2[:], u)
                gdt = f32 if it == 3 else bf16
                gg = small.tile([P, NSH], gdt, tag="gg%d" % it)
                with nc.allow_low_precision(reason="gain copy"):
                    if Rz is None:
                        nc.vector.tensor_mul(gg[:], g1[:], rden[:])
                    else:
                        g2 = small.tile([P, NSH], f32, tag="g2%d" % it)
                        nc.vector.tensor_mul(g2[:], g1[:], rden[:])
                        nc.vector.tensor_mul(gg[:], g2[:], Rz[:])
                odt = f32 if it == 3 else bf16
                OUT = work.tile([P, NK], odt, tag="out%d" % it)
                ow = nc.vector.tensor_mul(
                    OUT[:].rearrange("p (k n) -> p k n", n=NSH),
                    S[:].rearrange("p (k n) -> p k n", n=NSH),
                    gg[:].rearrange("p (o n) -> p o n", o=1)
                        .broadcast_to([P, DC, NSH]),
                )
                return OUT, ow

            # ---------------- iter 1 ----------------
            OUTr, _ = squash(psum_s1[0:B, 0:NK], 1)

            TMP = work.tile([128, CH, DC, NSH], bf16, tag="TMP")
            TREE = []
            for l in range(5):
                tl = work.tile([128, CH, DC // (2 ** (l + 1)), NSH], bf16,
                               tag="T%d" % l, name="T%d" % l)
                TREE.append(tl)
            Aprev = None
            SMUL = (3, 9, 10, 10)
            for it in (2, 3):
                # ---- a-step: TMP = OUTr*IH, tree-reduce k -> A [128,(c,n)]
                for h in range(2):
                    amul = nc.vector.tensor_mul(
                        TMP[:, h * 16:(h + 1) * 16]
                            .rearrange("p c k n -> p c (k n)"),
                        IH[:, h * 16:(h + 1) * 16]
                            .rearrange("p c k n -> p c (k n)"),
                        OUTr[:].rearrange("p (o f) -> p o f", o=1)
                              .broadcast_to([128, 16, NK]),
                    )
                    pace(amul)
                src = TMP
                for l in range(5):
                    half = DC // (2 ** (l + 1))
                    tadd = nc.vector.tensor_add(
                        TREE[l][:], src[:, :, 0:half, :],
                        src[:, :, half:2 * half, :])
                    if l in (0, 2, 4):
                        pace(tadd)
                    src = TREE[l]
                A = work.tile([128, CH, 1, NSH], bf16, tag="A%d" % it)
                nc.vector.tensor_add(A[:], src[:, :, 0:1, :],
                                     src[:, :, 1:2, :])
                if Aprev is None:
                    BL = A
                    Aprev = A
                else:
                    BL = work.tile([128, CH, 1, NSH], bf16, tag="BL")
                    nc.vector.tensor_add(BL[:], A[:], Aprev[:])
                # ---- E = exp(BL) on ACT (table resident)
                E = work.tile([128, CH, NSH], bf16, tag="E%d" % it)
                eact = nc.scalar.activation(
                    E[:], BL[:].rearrange("p c o n -> p c (o n)"), AF.Exp)
                pace(eact)
                # ---- s-step: TMP = E*IH per chunk, PE accumulates BD4^T TMP
                absorb("v", E[:2, 0, :2])  # chunk-0 mul keeps 1 wait (TMP WAR)
                pS = psb
                c0 = 0
                for gi, csz in enumerate(SMUL):
                    nc.vector.tensor_mul(
                        TMP[:, c0:c0 + csz],
                        IH[:, c0:c0 + csz],
                        E[:, c0:c0 + csz]
                          .rearrange("p c (o n) -> p c o n", o=1)
                          .broadcast_to([128, csz, DC, NSH]),
                    )
                    if gi == 0:
                        # Zp[p, n] = sum_c E  (before chunk-0 mms claim PE)
                        Zp = small.tile([128, NSH], bf16, tag="Zp")
                        with nc.allow_low_precision(reason="sum of positives"):
                            nc.vector.tensor_reduce(
                                Zp[:], E[:].rearrange("p c n -> p n c"),
                                axis=AX, op=ADD)
                    for c in range(c0, c0 + csz):
                        nc.tensor.matmul(
                            pS[:], bd4_t,
                            TMP[:, c].rearrange("p k n -> p (k n)"),
                            start=(c == 0), stop=False,
                            skip_group_check=True,
                        )
                    if gi == 0:
                        # Z fold on the PE, replicated for it==2 (G4) or
                        # 32-partition for it==3 (bd4)
                        if it < 3:
                            pzap = prep[:, 384:390]
                            pzmm = nc.tensor.matmul(
                                pzap, g4_t, Zp[:],
                                start=True, stop=True, skip_group_check=True)
                            PZ = 128
                        else:
                            pzap = psum_s1[0:B, 384:390]
                            pzmm = nc.tensor.matmul(
                                pzap, bd4_t, Zp[:],
                                start=True, stop=True, skip_group_check=True)
                            PZ = B
                        ZB = work.tile([128, NK], bf16, tag="ZB%d" % it)
                        zb = nc.vector.tensor_mul(
                            ZB[:].rearrange("p (k n) -> p k n", n=NSH),
                            brepR_t.rearrange("p (k n) -> p k n", n=NSH),
                            Zp[:].rearrange("p (o n) -> p o n", o=1)
                                .broadcast_to([128, DC, NSH]),
                        )
                    elif gi == 1:
                        Zs = small.tile([PZ, NSH], f32, tag="Zs%d" % it)
                        nc.vector.tensor_copy(Zs[:], pzap)
                        Rz = small.tile([PZ, NSH], f32, tag="Rz%d" % it)
                        nc.vector.reciprocal(Rz[:], Zs[:])
                        Rz2 = small.tile([PZ, NSH], f32, tag="Rz2%d" % it)
                        nc.vector.tensor_mul(Rz2[:], Rz[:], Rz[:])
                    c0 += csz
                # ZB closes the accumulation group
                mm_last = nc.tensor.matmul(pS[:], bd4_t, ZB[:],
                                           start=False, stop=True,
                                           skip_group_check=True)
                add_dep_helper(mm_last.ins, zb.ins, sync=True,
                               reason="ZB matmul waits ZB mul")
                OUT, out_w = squash(pS[:], it, Rz=Rz, Rz2=Rz2)
                if it < 3:
                    OUTr = OUT
                else:
                    absorb("s", OUT[:2, :2])
                    o_dma = nc.scalar.dma_start(out=out_d[:], in_=OUT[:])
                    f_scr = small.tile([2, 4], f32, tag="fin")
                    f_act = nc.scalar.copy(f_scr[:, 0:2], OUT[:2, :2])
                    f_dve = nc.vector.tensor_copy(f_scr[:, 2:4], OUT[:2, :2])
                    f_pe = pace(out_w)
                    for fin in (cb_dma, *s_dmas, mm_last, mm_s1,
                                zb, f_act, f_dve, f_pe, o_dma):
                        fnop = nc.sync.nop()
                        add_dep_helper(fnop.ins, fin.ins, sync=True,
                                       reason="absorb final sem for drain")

    return nc


def _pack_inputs(inputs, W, B_param):
    bf = ml_dtypes.bfloat16
    w8 = ml_dtypes.float8_e3m4 if FP8W else bf
    inputs = np.ascontiguousarray(inputs, dtype=np.float32)
    W = np.ascontiguousarray(W, dtype=np.float32)
    B_param = np.ascontiguousarray(B_param, dtype=np.float32)

    Wp = np.zeros((CH, NCP, DC, DIN), dtype=np.float32)
    Wp[:, :NC] = W
    Bp = np.zeros((NCP, DC), dtype=np.float32)
    Bp[:NC] = B_param

    # xt[c, dc, dd, (b,rr)] = x[b, 4c+rr, 128dc+dd]
    x4 = inputs.reshape(B, CH, 4, 2, 128)            # b c rr dc dd
    xt = np.ascontiguousarray(
        x4.transpose(1, 3, 4, 0, 2)).reshape(CH, 2, 128, 128)
    bd4 = np.zeros((128, B), dtype=np.float32)
    bd4[np.arange(128), np.arange(128) // 4] = 1.0
    g4 = np.zeros((128, 128), dtype=np.float32)
    g4[np.arange(128)[:, None] // 4 == np.arange(128)[None, :] // 4] = 1.0

    in_maps = []
    for core in range(NCORES):
        sl = slice(core * NSH, (core + 1) * NSH)
        # wt[c, dc, dd, (k, n)] = W[c, n, k, 128dc+dd]
        w5 = Wp[:, sl].reshape(CH, NSH, DC, 2, 128)  # c n k dc dd
        wt = np.ascontiguousarray(
            w5.transpose(0, 3, 4, 2, 1)).reshape(CH, 2, 128, NK)
        if FP8W:
            amax = np.abs(wt).reshape(CH, -1).max(axis=1)
            sw = 15.0 / np.maximum(amax, 1e-30)
        else:
            sw = np.ones(CH, dtype=np.float32)
        wt_q = (wt * sw[:, None, None, None]).astype(w8)
        xt_c = (xt / sw[:, None, None, None]).astype(bf)
        # merged byte stream [c, dd, xt0|xt1|wt0|wt1], chunk-contiguous
        RB = 512 + (768 if FP8W else 1536)
        WBY = 384 if FP8W else 768
        sb = np.zeros((CH, 128, RB), dtype=np.uint8)
        xb = np.ascontiguousarray(xt_c.transpose(0, 2, 1, 3))  # c dd dc br
        sb[:, :, 0:512] = xb.view(np.uint8).reshape(CH, 128, 512)
        wb = np.ascontiguousarray(wt_q.transpose(0, 2, 1, 3))  # c dd dc kn
        sb[:, :, 512:RB] = wb.view(np.uint8).reshape(CH, 128, 2 * WBY)
        sdc = np.ascontiguousarray(
            sb.reshape(NCHUNK, CPC, 128, RB).transpose(0, 2, 1, 3)
        ).reshape(NCHUNK, 128, CPC * RB)
        brep = np.ascontiguousarray(Bp[sl].T).reshape(1, NK)  # (k, n) flat
        cstb = np.zeros((128, 672), dtype=np.float32)
        cstb[:, 0:B] = bd4
        cstb[0:B, B:B + 128] = bd4.T
        cstb[:, 160:288] = g4
        cstb[:, 288:288 + NK] = brep
        in_maps.append(dict(sd=sdc, cstb=cstb.astype(bf)))
    return in_maps


def _run(inputs, W, B_param, trace=False):
    from concourse.bass_utils import run_bass_kernel_spmd

    if "nc" not in _cache:
        _cache["nc"] = _build_nc()
    nc = _cache["nc"]
    in_maps = _pack_inputs(inputs, W, B_param)
    res = run_bass_kernel_spmd(nc, in_maps, core_ids=list(range(NCORES)),
                               trace=trace)
    # out[b, (k, n)] -> [b, n, k]
    outs = [r["out"].reshape(B, DC, NSH).transpose(0, 2, 1)
            for r in res.results]
    full = np.concatenate(outs, axis=1)[:, :NC, :]
    return np.ascontiguousarray(full.astype(np.float32)), res


def kernel(inputs, W, B_param):
    out, _ = _run(inputs, W, B_param, trace=False)
    return out
